# revision 12
# baseline (speedup 1.0000x reference)
"""Trainium2 Bass kernel for CheemsNonWoAttn (GQA attention block, no Wo).

Sharding: 8 cores = batch(2) x kv-head-pair(4). Each core handles one batch
element and 2 of the 8 kv heads (GQA: Q and K are repeated identically across
the 4 groups, so only 8 unique softmax matrices exist; V uses all 32 heads).

Per-core device program — 4-chunk pipeline over the sequence (chunks of 512):
  for each chunk c: Qt/Kt = Wq^T X_c^T, Wk^T X_c^T (PSUM), RoPE on DVE,
  V_c = X_c Wv, scores^T[k,q] = Kt^T Qt per head (row-half packed),
  E = exp(scores) on ACT, causal diag masked on DVE,
  out = E^T V with a ones-column producing the softmax denominator,
  denominator division staged through SBUF (ACT copy, DVE recip+scale).
Emission interleaves score tiles with projection matmuls so the PE stream
never stalls >~2us (keeps the HAM clock gate at 8/8 = 2.4 GHz).
"""
import os
import sys
import types
from contextlib import ExitStack

for _p in ("/opt/trn_rl_repo", "/root/.axon_site/_ro/trn_rl_repo"):
    if os.path.isdir(_p) and _p not in sys.path:
        sys.path.append(_p)

import numpy as np
import ml_dtypes

import concourse.bass as bass
import concourse.tile as tile
from concourse import mybir
from concourse.bass_utils import run_bass_kernel_spmd
from concourse.vector_clock import ScopedClock

# ---------------------------------------------------------------------------
# Patch 1: walrus rejects Drain instructions with >1 sync wait (CTRL ops have
# a single wait slot). Split the TileContext exit drain's waits across extra
# SP nops, one wait each.
def _patched_drain_and_barrier(self, tick_clock, wait_clock):
    nc = self.nc
    drain_bi = nc.sync.drain()
    wait_clock.add_sem_waits(drain_bi.ins, ScopedClock({None: tick_clock.global_clock}))
    inst = drain_bi.ins
    si = inst.sync_info
    if si is not None and si.on_wait is not None and len(si.on_wait) > 1:
        waits = list(si.on_wait)
        inst.sync_info = mybir.SyncInfo(
            on_wait=waits[:1],
            on_update=list(si.on_update) if si.on_update else [],
        )
        for w in waits[1:]:
            nbi = nc.sync.nop()
            nbi.ins.sync_info = mybir.SyncInfo(on_wait=[w], on_update=[])
    nc.all_engine_barrier()
    assert self.sems is not None
    popped = nc._tile_sem_poison_stack.pop()
    assert popped is self._sem_poison
    nc.clear_and_free_semaphores(list(self.sems.allocated().values()))
    nc.all_engine_barrier()


tile.TileContext._drain_and_barrier = _patched_drain_and_barrier


def _legalize_waits(nc):
    """This walrus build accepts at most one sync-wait per instruction.
    Split any instruction carrying N>1 waits into N-1 preceding same-engine
    nops (engines are in-order, so semantics are preserved)."""
    uid = 0
    for f in nc.m.functions:
        for blk in f.blocks:
            insts = list(blk.instructions)
            out, changed = [], False
            for inst in insts:
                si = getattr(inst, "sync_info", None)
                if si is not None and si.on_wait is not None and len(si.on_wait) > 1:
                    waits = list(si.on_wait)
                    for w in waits[:-1]:
                        uid += 1
                        out.append(mybir.InstNoOp(
                            name=f"{inst.name}_lw{uid}",
                            engine=inst.engine,
                            sync_info=mybir.SyncInfo(on_wait=[w], on_update=[]),
                            bass_nofuse=True,
                        ))
                    inst.sync_info = mybir.SyncInfo(
                        on_wait=waits[-1:],
                        on_update=list(si.on_update) if si.on_update else [],
                    )
                    changed = True
                out.append(inst)
            if changed:
                blk.instructions = out


def _dedup_ldweights(nc):
    """Clear the self-load on matmuls whose stationary operand AP is identical
    to the immediately preceding matmul in the final PE stream (walrus runs
    with ldw-opt disabled, so it reloads weights for every matmul otherwise)."""
    if os.environ.get("CHEEMS_NO_LDW_DEDUP"):
        return
    for f in nc.m.functions:
        for blk in f.blocks:
            prev_key = None
            for inst in blk.instructions:
                if not isinstance(inst, mybir.InstMatmult):
                    if isinstance(inst, mybir.InstLdweights):
                        prev_key = None
                    continue
                key = (repr(inst.ins[1]), inst.perf_mode, inst.is_transpose,
                       repr(inst.tile_position))
                if prev_key is not None and key == prev_key:
                    inst.ldweights = False
                prev_key = key


# Patch 2 (optional, for tracing): recreate the antenv.axon_hooks shim so
# run_bass_kernel_spmd(trace=True) can capture NTFF profiles under axon.
def _install_ntff_hook():
    try:
        if "antenv.axon_hooks" in sys.modules:
            return
        import antenv
        from trn_agent_boot.trn_boot import _ntff_profile_via_ctypes

        hook = _ntff_profile_via_ctypes("/opt/axon/libaxon_pjrt.so")
        mod = types.ModuleType("antenv.axon_hooks")
        mod._hook = hook
        mod.get_axon_ntff_profile_hook = lambda: mod._hook

        def _set(h):
            mod._hook = h

        mod.set_axon_ntff_profile_hook = _set
        sys.modules["antenv.axon_hooks"] = mod
        antenv.axon_hooks = mod
    except Exception:
        pass


# ---------------------------------------------------------------------------
B, S, HID = 2, 2048, 2048
NH, G = 32, 4
HD = 64          # head dim
HKV = 8          # kv heads
THETA = 10000.0
P = 128          # partitions
NKT = HID // P   # 16 k-tiles over the contraction dim
NST = S // P     # 16 s-tiles
NCH = 4          # s-chunks of 512
CH = 512
DV = 512         # v columns per core (8 heads x 64)
VROW = 528       # v tile row: [256 v | 1 one | 7 pad] x 2 heads
VOFF = 264

F32 = mybir.dt.float32
BF16 = mybir.dt.bfloat16

_CACHE = {}
LAST_RESULTS = None


def _build():
    nc = bass.Bass("TRN2")
    d_xt = nc.declare_dram_parameter("xt", [P, NKT, S], BF16, isOutput=False)
    d_wq = nc.declare_dram_parameter("wq", [P, NKT, P], BF16, isOutput=False)
    d_wk = nc.declare_dram_parameter("wk", [P, NKT, P], BF16, isOutput=False)
    d_wv = nc.declare_dram_parameter("wv", [P, NKT, DV], BF16, isOutput=False)
    d_cos = nc.declare_dram_parameter("cost", [P, S], BF16, isOutput=False)
    d_sin = nc.declare_dram_parameter("sint", [P, S], BF16, isOutput=False)
    d_tri = nc.declare_dram_parameter("tri", [P, P], BF16, isOutput=False)
    d_out = nc.declare_dram_parameter("out", [S, 2, 257], BF16, isOutput=True)

    with tile.TileContext(nc) as tc, ExitStack() as ctx:
        pers = ctx.enter_context(tc.tile_pool(name="pers", bufs=1))
        xpool = ctx.enter_context(tc.tile_pool(name="xpool", bufs=3))
        epool = ctx.enter_context(tc.tile_pool(name="epool", bufs=48))
        work = ctx.enter_context(tc.tile_pool(name="work", bufs=2))
        outp = ctx.enter_context(tc.tile_pool(name="outp", bufs=3))
        psum = ctx.enter_context(tc.tile_pool(name="psum", bufs=8, space="PSUM"))

        # --- persistent tiles
        wq_sb = pers.tile([P, NKT, P], BF16, tag="wq")
        wk_sb = pers.tile([P, NKT, P], BF16, tag="wk")
        wv_sb = pers.tile([P, NKT, DV], BF16, tag="wv")
        cos_sb = pers.tile([P, S], BF16, tag="cos")
        sin_sb = pers.tile([P, S], BF16, tag="sin")
        tri_sb = pers.tile([P, P], BF16, tag="tri")
        qt = pers.tile([P, S], BF16, tag="qt")
        kt = pers.tile([P, S], BF16, tag="kt")
        v_sb = [pers.tile([P, VROW], BF16, tag=f"v{t}", name=f"v{t}") for t in range(NST)]

        xt = [xpool.tile([P, NKT, CH], BF16, tag="xt", name=f"xt{c}") for c in range(NCH)]

        # --- DMA emission order (= data arrival order; see module docstring)
        # first pieces are tiny so the first matmuls start ~2us earlier,
        # interleaved so the QK k-loop is never starved of weights or xt
        nc.sync.dma_start(out=wq_sb[:, 0:2, :], in_=d_wq[:, 0:2, :])
        nc.sync.dma_start(out=xt[0][:, 0:2, :], in_=d_xt[:, 0:2, bass.ts(0, CH)])
        nc.sync.dma_start(out=wk_sb[:, 0:2, :], in_=d_wk[:, 0:2, :])
        nc.sync.dma_start(out=xt[0][:, 2:4, :], in_=d_xt[:, 2:4, bass.ts(0, CH)])
        nc.sync.dma_start(out=wq_sb[:, 2:8, :], in_=d_wq[:, 2:8, :])
        nc.sync.dma_start(out=wk_sb[:, 2:8, :], in_=d_wk[:, 2:8, :])
        nc.sync.dma_start(out=xt[0][:, 4:8, :], in_=d_xt[:, 4:8, bass.ts(0, CH)])
        nc.sync.dma_start(out=wq_sb[:, 8:16, :], in_=d_wq[:, 8:16, :])
        nc.sync.dma_start(out=wk_sb[:, 8:16, :], in_=d_wk[:, 8:16, :])
        nc.sync.dma_start(out=xt[0][:, 8:12, :], in_=d_xt[:, 8:12, bass.ts(0, CH)])
        nc.sync.dma_start(out=xt[0][:, 12:16, :], in_=d_xt[:, 12:16, bass.ts(0, CH)])
        nc.sync.dma_start(out=cos_sb[:], in_=d_cos[:])
        nc.sync.dma_start(out=sin_sb[:], in_=d_sin[:])
        nc.sync.dma_start(out=tri_sb[:], in_=d_tri[:])
        for kk in range(4):
            nc.sync.dma_start(out=wv_sb[:, bass.ts(kk, 4), :],
                              in_=d_wv[:, bass.ts(kk, 4), :])
        for c in range(1, NCH):
            for kk in range(4):
                nc.sync.dma_start(out=xt[c][:, bass.ts(kk, 4), :],
                                  in_=d_xt[:, bass.ts(kk, 4), bass.ts(c, CH)])

        # --- emit helpers -------------------------------------------------
        def rope(src_ps, dst, c):
            # ACT copies the PSUM to bf16 SBUF (frees the bank fast), then the
            # rotate/scale runs all-bf16 on DVE at the 2x 16-bit rate
            cs = bass.ts(c, CH)
            raw = work.tile([P, CH], BF16, tag="raw", name="raw")
            nc.scalar.activation(raw[:], src_ps[:], mybir.ActivationFunctionType.Copy)
            t1 = work.tile([P, CH], BF16, tag="t1", name="t1")
            t2 = work.tile([P, CH], BF16, tag="t2", name="t2")
            nc.vector.tensor_mul(out=t1[:], in0=raw[:], in1=cos_sb[:, cs])
            for blk in range(4):
                lo = blk * 32
                swap_lo = (blk ^ 1) * 32
                # in0 stays in PSUM: cross-partition-offset operands are only
                # legal when one input is in PSUM (walrus SB-SB check)
                nc.vector.tensor_mul(out=t2[lo:lo + 32, :],
                                     in0=src_ps[swap_lo:swap_lo + 32, :],
                                     in1=sin_sb[lo:lo + 32, cs])
            nc.vector.tensor_add(out=dst[:, cs], in0=t1[:], in1=t2[:])

        pq_t, pk_t = {}, {}

        def qk_mms(c, k0, k1):
            if c not in pq_t:
                pq_t[c] = psum.tile([P, CH], F32, tag="pq", bufs=1, name=f"pq{c}")
                pk_t[c] = psum.tile([P, CH], F32, tag="pk", bufs=1, name=f"pk{c}")
            pq, pk = pq_t[c], pk_t[c]
            for k in range(k0, k1):
                nc.tensor.matmul(pq[:], lhsT=wq_sb[:, k, :], rhs=xt[c][:, k, :],
                                 start=(k == 0), stop=(k == NKT - 1), skip_group_check=True)
                nc.tensor.matmul(pk[:], lhsT=wk_sb[:, k, :], rhs=xt[c][:, k, :],
                                 start=(k == 0), stop=(k == NKT - 1), skip_group_check=True)
            if k1 == NKT:
                rope(pq, qt, c)
                rope(pk, kt, c)

        def v_fin(pv, t):
            nc.vector.tensor_copy(out=v_sb[t][:, 0:256], in_=pv[:, 0:256])
            nc.vector.tensor_copy(out=v_sb[t][:, VOFF:VOFF + 256], in_=pv[:, 256:512])
            nc.vector.memset(v_sb[t][:, 256:257], 1.0)
            nc.vector.memset(v_sb[t][:, VOFF + 256:VOFF + 257], 1.0)

        def v_single(t):
            c, tt = t // 4, t % 4
            pv = psum.tile([P, DV], F32, tag="pv", bufs=2, name=f"pv{t}")
            for k in range(NKT):
                nc.tensor.matmul(pv[:], lhsT=xt[c][:, k, bass.ts(tt, P)],
                                 rhs=wv_sb[:, k, :],
                                 start=(k == 0), stop=(k == NKT - 1), skip_group_check=True)
            v_fin(pv, t)

        def v_pair(c, tt0):
            v_single(4 * c + tt0)
            v_single(4 * c + tt0 + 1)

        e_tiles = {}

        def score_tile(c, t):
            cs0 = c * CH
            m = t - 4 * c
            off = max(m, 0) * P
            w = CH - off
            # both heads in one 2-bank PSUM tile: the pair of K=64 matmuls
            # lands adjacent (disjoint row halves -> concurrent in the array)
            # and a single paired exp halves the ACT per-op overhead
            ps_s = psum.tile([P, 2, CH], F32, tag="ps", bufs=1, name="ps_s")
            for h in range(2):
                nc.tensor.matmul(
                    ps_s[:, h, 0:w],
                    lhsT=kt[h * HD:(h + 1) * HD, bass.ts(t, P)],
                    rhs=qt[h * HD:(h + 1) * HD, bass.ds(cs0 + off, w)],
                    start=True, stop=True, skip_group_check=True)
            e = epool.tile([P, 2, CH], BF16, tag="e", bufs=24, name=f"e{c}_{t}")
            nc.scalar.activation(e[:, :, bass.ds(off, w)], ps_s[:, :, 0:w],
                                 mybir.ActivationFunctionType.Exp)
            if m >= 0:
                for h in range(2):
                    nc.vector.tensor_mul(out=e[:, h, bass.ts(m, P)],
                                         in0=e[:, h, bass.ts(m, P)], in1=tri_sb[:])
            e_tiles[(c, t)] = e

        def av_m(c, m):
            q_idx = 4 * c + m
            stage = outp.tile([P, 2, 257], BF16, tag="stage", name="stage")
            for h in range(2):
                po = psum.tile([P, CH], F32, tag="po", bufs=2, name="po")
                for t in range(q_idx + 1):
                    nc.tensor.matmul(
                        po[:, 0:257],
                        lhsT=e_tiles[(c, t)][:, h, bass.ts(m, P)],
                        rhs=v_sb[t][:, h * VOFF:h * VOFF + 257],
                        start=(t == 0), stop=(t == q_idx), skip_group_check=True)
                # stage raw numerator+denominator through SBUF; the softmax
                # division runs on host. ACT is free in the early chunks,
                # DVE in the late ones (ACT is then busy with exps).
                if c < 2:
                    nc.scalar.activation(stage[:, h, :], po[:, 0:257],
                                         mybir.ActivationFunctionType.Copy)
                else:
                    nc.vector.tensor_copy(out=stage[:, h, :], in_=po[:, 0:257])
            nc.sync.dma_start(out=d_out[bass.ts(q_idx, P), :, :], in_=stage[:])

        # --- main emission sequence --------------------------------------
        qk_mms(0, 0, NKT)
        v_pair(0, 0)
        v_pair(0, 2)
        score_tile(0, 0)
        score_tile(0, 1)
        qk_mms(1, 0, 8)
        score_tile(0, 2)
        score_tile(0, 3)
        qk_mms(1, 8, NKT)
        for mm_ in range(4):
            av_m(0, mm_)
        v_pair(1, 0)
        v_pair(1, 2)
        qk_mms(2, 0, NKT)
        # scores c1 interleaved with av c1 (ACT-paced region)
        for t in range(5):
            score_tile(1, t)
        qk_mms(3, 0, 8)
        score_tile(1, 5)
        score_tile(1, 6)
        score_tile(1, 7)
        qk_mms(3, 8, NKT)
        av_m(1, 0)
        av_m(1, 1)
        av_m(1, 2)
        av_m(1, 3)
        # scores c2 interleaved with V chunk 2
        score_tile(2, 0)
        score_tile(2, 1)
        score_tile(2, 2)
        v_pair(2, 0)
        score_tile(2, 3)
        score_tile(2, 4)
        score_tile(2, 5)
        v_pair(2, 2)
        for t in range(6, 12):
            score_tile(2, t)
        # scores c3 (t0-11) interleaved with av c2
        score_tile(3, 0)
        score_tile(3, 1)
        score_tile(3, 2)
        av_m(2, 0)
        score_tile(3, 3)
        score_tile(3, 4)
        score_tile(3, 5)
        av_m(2, 1)
        score_tile(3, 6)
        score_tile(3, 7)
        score_tile(3, 8)
        av_m(2, 2)
        score_tile(3, 9)
        score_tile(3, 10)
        score_tile(3, 11)
        av_m(2, 3)
        # tail: V chunk 3 tiles interleaved with diag scores + av c3
        v_single(12)
        score_tile(3, 12)
        av_m(3, 0)
        v_single(13)
        score_tile(3, 13)
        av_m(3, 1)
        v_single(14)
        score_tile(3, 14)
        av_m(3, 2)
        v_single(15)
        score_tile(3, 15)
        av_m(3, 3)

    _legalize_waits(nc)
    _dedup_ldweights(nc)
    return nc


def _host_prep(hidden_states, position_ids, Wq, Wk, Wv):
    """Build the 8 per-core input maps."""
    hidden_states = np.asarray(hidden_states, dtype=np.float32)
    position_ids = np.asarray(position_ids)
    Wq = np.asarray(Wq, dtype=np.float32)
    Wk = np.asarray(Wk, dtype=np.float32)
    Wv = np.asarray(Wv, dtype=np.float32)

    scale = 1.0 / np.sqrt(HD)
    tri = np.triu(np.ones((P, P), dtype=np.float32)).astype(ml_dtypes.bfloat16)
    inv_freq = (1.0 / (THETA ** (np.arange(0, HD, 2, dtype=np.float32) / HD))).astype(np.float32)

    def _pkt(a):  # [HID, N] -> [P, NKT, N] (partition-major k-tiles)
        return np.ascontiguousarray(
            a.reshape(NKT, P, a.shape[1]).transpose(1, 0, 2))

    in_maps = []
    for core in range(8):
        b, p = core // 4, core % 4
        xt = _pkt(np.ascontiguousarray(hidden_states[b].T)).astype(ml_dtypes.bfloat16)
        wq = _pkt(Wq[:, p * P:(p + 1) * P] * scale).astype(ml_dtypes.bfloat16)
        wk = _pkt(Wk[:, p * P:(p + 1) * P]).astype(ml_dtypes.bfloat16)
        cols = []
        for h in (2 * p, 2 * p + 1):
            for r in range(G):
                j = r * HKV + h
                cols.append(Wv[:, j * HD:(j + 1) * HD])
        wv = _pkt(np.concatenate(cols, axis=1)).astype(ml_dtypes.bfloat16)

        pos = position_ids[b].astype(np.float32)
        freqs = pos[:, None] * inv_freq[None, :]          # [S, 32]
        cos32 = np.cos(freqs).T.astype(np.float32)        # [32, S]
        sin32 = np.sin(freqs).T.astype(np.float32)
        cost = np.ascontiguousarray(np.tile(cos32, (4, 1))).astype(ml_dtypes.bfloat16)
        sint = np.ascontiguousarray(
            np.tile(np.concatenate([-sin32, sin32], axis=0), (2, 1))).astype(ml_dtypes.bfloat16)

        in_maps.append({
            "xt": xt, "wq": wq, "wk": wk, "wv": wv,
            "cost": cost, "sint": sint, "tri": tri,
        })
    return in_maps


def kernel(hidden_states, position_ids, Wq, Wk, Wv):
    global LAST_RESULTS
    trace = bool(os.environ.get("CHEEMS_TRACE"))
    if trace:
        _install_ntff_hook()
    if "nc" not in _CACHE:
        _CACHE["nc"] = _build()
    nc = _CACHE["nc"]
    in_maps = _host_prep(hidden_states, position_ids, Wq, Wk, Wv)
    res = run_bass_kernel_spmd(nc, in_maps, core_ids=list(range(8)), trace=trace)
    LAST_RESULTS = res

    out = np.empty((B, S, HID), dtype=np.float32)
    for core in range(8):
        b, p = core // 4, core % 4
        raw = res.results[core]["out"].astype(np.float32)        # [S, 2, 257]
        core_out = raw[:, :, 0:256] / raw[:, :, 256:257]         # softmax denom
        for hl, h in enumerate((2 * p, 2 * p + 1)):
            for r in range(G):
                j = r * HKV + h
                out[b, :, j * HD:(j + 1) * HD] = core_out[:, hl, r * HD:(r + 1) * HD]
    return out.reshape(B, S, HID)


# revision 15
# speedup vs baseline: 1.0317x; 1.0317x over previous
"""Trainium2 Bass kernel for CheemsNonWoAttn (GQA attention block, no Wo).

Sharding: 8 cores = batch(2) x kv-head-pair(4). Each core handles one batch
element and 2 of the 8 kv heads (GQA: Q and K are repeated identically across
the 4 groups, so only 8 unique softmax matrices exist; V uses all 32 heads).

Per-core device program — 4-chunk pipeline over the sequence (chunks of 512):
  for each chunk c: Qt/Kt = Wq^T X_c^T, Wk^T X_c^T (PSUM), RoPE on DVE,
  V_c = X_c Wv, scores^T[k,q] = Kt^T Qt per head (row-half packed),
  E = exp(scores) on ACT, causal diag masked on DVE,
  out = E^T V with a ones-column producing the softmax denominator,
  denominator division staged through SBUF (ACT copy, DVE recip+scale).
Emission interleaves score tiles with projection matmuls so the PE stream
never stalls >~2us (keeps the HAM clock gate at 8/8 = 2.4 GHz).
"""
import os
import sys
import types
from contextlib import ExitStack

for _p in ("/opt/trn_rl_repo", "/root/.axon_site/_ro/trn_rl_repo"):
    if os.path.isdir(_p) and _p not in sys.path:
        sys.path.append(_p)

import numpy as np
import ml_dtypes

import concourse.bass as bass
import concourse.tile as tile
from concourse import mybir
from concourse.bass_utils import run_bass_kernel_spmd
from concourse.vector_clock import ScopedClock

# ---------------------------------------------------------------------------
# Patch 1: walrus rejects Drain instructions with >1 sync wait (CTRL ops have
# a single wait slot). Split the TileContext exit drain's waits across extra
# SP nops, one wait each.
def _patched_drain_and_barrier(self, tick_clock, wait_clock):
    nc = self.nc
    drain_bi = nc.sync.drain()
    wait_clock.add_sem_waits(drain_bi.ins, ScopedClock({None: tick_clock.global_clock}))
    inst = drain_bi.ins
    si = inst.sync_info
    if si is not None and si.on_wait is not None and len(si.on_wait) > 1:
        waits = list(si.on_wait)
        inst.sync_info = mybir.SyncInfo(
            on_wait=waits[:1],
            on_update=list(si.on_update) if si.on_update else [],
        )
        for w in waits[1:]:
            nbi = nc.sync.nop()
            nbi.ins.sync_info = mybir.SyncInfo(on_wait=[w], on_update=[])
    nc.all_engine_barrier()
    assert self.sems is not None
    popped = nc._tile_sem_poison_stack.pop()
    assert popped is self._sem_poison
    nc.clear_and_free_semaphores(list(self.sems.allocated().values()))
    nc.all_engine_barrier()


tile.TileContext._drain_and_barrier = _patched_drain_and_barrier


def _legalize_waits(nc):
    """This walrus build accepts at most one sync-wait per instruction.
    Split any instruction carrying N>1 waits into N-1 preceding same-engine
    nops (engines are in-order, so semantics are preserved)."""
    uid = 0
    for f in nc.m.functions:
        for blk in f.blocks:
            insts = list(blk.instructions)
            out, changed = [], False
            for inst in insts:
                si = getattr(inst, "sync_info", None)
                if si is not None and si.on_wait is not None and len(si.on_wait) > 1:
                    waits = list(si.on_wait)
                    for w in waits[:-1]:
                        uid += 1
                        out.append(mybir.InstNoOp(
                            name=f"{inst.name}_lw{uid}",
                            engine=inst.engine,
                            sync_info=mybir.SyncInfo(on_wait=[w], on_update=[]),
                            bass_nofuse=True,
                        ))
                    inst.sync_info = mybir.SyncInfo(
                        on_wait=waits[-1:],
                        on_update=list(si.on_update) if si.on_update else [],
                    )
                    changed = True
                out.append(inst)
            if changed:
                blk.instructions = out


def _dedup_ldweights(nc):
    """Clear the self-load on matmuls whose stationary operand AP is identical
    to the immediately preceding matmul in the final PE stream (walrus runs
    with ldw-opt disabled, so it reloads weights for every matmul otherwise)."""
    if os.environ.get("CHEEMS_NO_LDW_DEDUP"):
        return
    for f in nc.m.functions:
        for blk in f.blocks:
            prev_key = None
            for inst in blk.instructions:
                if not isinstance(inst, mybir.InstMatmult):
                    if isinstance(inst, mybir.InstLdweights):
                        prev_key = None
                    continue
                key = (repr(inst.ins[1]), inst.perf_mode, inst.is_transpose,
                       repr(inst.tile_position))
                if prev_key is not None and key == prev_key:
                    inst.ldweights = False
                prev_key = key


# Patch 2 (optional, for tracing): recreate the antenv.axon_hooks shim so
# run_bass_kernel_spmd(trace=True) can capture NTFF profiles under axon.
def _install_ntff_hook():
    try:
        if "antenv.axon_hooks" in sys.modules:
            return
        import antenv
        from trn_agent_boot.trn_boot import _ntff_profile_via_ctypes

        hook = _ntff_profile_via_ctypes("/opt/axon/libaxon_pjrt.so")
        mod = types.ModuleType("antenv.axon_hooks")
        mod._hook = hook
        mod.get_axon_ntff_profile_hook = lambda: mod._hook

        def _set(h):
            mod._hook = h

        mod.set_axon_ntff_profile_hook = _set
        sys.modules["antenv.axon_hooks"] = mod
        antenv.axon_hooks = mod
    except Exception:
        pass


# ---------------------------------------------------------------------------
B, S, HID = 2, 2048, 2048
NH, G = 32, 4
HD = 64          # head dim
HKV = 8          # kv heads
THETA = 10000.0
P = 128          # partitions
NKT = HID // P   # 16 k-tiles over the contraction dim
NST = S // P     # 16 s-tiles
NCH = 4          # s-chunks of 512
CH = 512
DV = 512         # v columns per core (8 heads x 64)
VROW = 528       # v tile row: [256 v | 1 one | 7 pad] x 2 heads
VOFF = 264

F32 = mybir.dt.float32
BF16 = mybir.dt.bfloat16

_CACHE = {}
LAST_RESULTS = None


def _build():
    nc = bass.Bass("TRN2")
    d_xt = nc.declare_dram_parameter("xt", [P, NKT, S], BF16, isOutput=False)
    d_wq = nc.declare_dram_parameter("wq", [P, NKT, P], BF16, isOutput=False)
    d_wk = nc.declare_dram_parameter("wk", [P, NKT, P], BF16, isOutput=False)
    d_wv = nc.declare_dram_parameter("wv", [P, NKT, DV], BF16, isOutput=False)
    d_cos = nc.declare_dram_parameter("cost", [P, S], BF16, isOutput=False)
    d_sin = nc.declare_dram_parameter("sint", [P, S], BF16, isOutput=False)
    d_tri = nc.declare_dram_parameter("tri", [P, P], BF16, isOutput=False)
    d_out = nc.declare_dram_parameter("out", [S, 2, 257], BF16, isOutput=True)

    with tile.TileContext(nc) as tc, ExitStack() as ctx:
        pers = ctx.enter_context(tc.tile_pool(name="pers", bufs=1))
        xpool = ctx.enter_context(tc.tile_pool(name="xpool", bufs=3))
        epool = ctx.enter_context(tc.tile_pool(name="epool", bufs=48))
        work = ctx.enter_context(tc.tile_pool(name="work", bufs=2))
        outp = ctx.enter_context(tc.tile_pool(name="outp", bufs=3))
        psum = ctx.enter_context(tc.tile_pool(name="psum", bufs=8, space="PSUM"))

        # --- persistent tiles
        wq_sb = pers.tile([P, NKT, P], BF16, tag="wq")
        wk_sb = pers.tile([P, NKT, P], BF16, tag="wk")
        wv_sb = pers.tile([P, NKT, DV], BF16, tag="wv")
        cos_sb = pers.tile([P, S], BF16, tag="cos")
        sin_sb = pers.tile([P, S], BF16, tag="sin")
        tri_sb = pers.tile([P, P], BF16, tag="tri")
        qt = pers.tile([P, S], BF16, tag="qt")
        kt = pers.tile([P, S], BF16, tag="kt")
        v_sb = [pers.tile([P, VROW], BF16, tag=f"v{t}", name=f"v{t}") for t in range(NST)]

        xt = [xpool.tile([P, NKT, CH], BF16, tag="xt", name=f"xt{c}") for c in range(NCH)]

        # --- DMA emission order (= data arrival order; see module docstring)
        # first pieces are tiny so the first matmuls start ~2us earlier,
        # interleaved so the QK k-loop is never starved of weights or xt
        nc.sync.dma_start(out=wq_sb[:, 0:2, :], in_=d_wq[:, 0:2, :])
        nc.sync.dma_start(out=xt[0][:, 0:2, :], in_=d_xt[:, 0:2, bass.ts(0, CH)])
        nc.sync.dma_start(out=wk_sb[:, 0:2, :], in_=d_wk[:, 0:2, :])
        nc.sync.dma_start(out=xt[0][:, 2:4, :], in_=d_xt[:, 2:4, bass.ts(0, CH)])
        nc.sync.dma_start(out=wq_sb[:, 2:8, :], in_=d_wq[:, 2:8, :])
        nc.sync.dma_start(out=wk_sb[:, 2:8, :], in_=d_wk[:, 2:8, :])
        nc.sync.dma_start(out=xt[0][:, 4:8, :], in_=d_xt[:, 4:8, bass.ts(0, CH)])
        nc.sync.dma_start(out=wq_sb[:, 8:16, :], in_=d_wq[:, 8:16, :])
        nc.sync.dma_start(out=wk_sb[:, 8:16, :], in_=d_wk[:, 8:16, :])
        nc.sync.dma_start(out=xt[0][:, 8:12, :], in_=d_xt[:, 8:12, bass.ts(0, CH)])
        nc.sync.dma_start(out=xt[0][:, 12:16, :], in_=d_xt[:, 12:16, bass.ts(0, CH)])
        nc.sync.dma_start(out=cos_sb[:], in_=d_cos[:])
        nc.sync.dma_start(out=sin_sb[:], in_=d_sin[:])
        nc.sync.dma_start(out=tri_sb[:], in_=d_tri[:])
        for kk in range(4):
            nc.sync.dma_start(out=wv_sb[:, bass.ts(kk, 4), :],
                              in_=d_wv[:, bass.ts(kk, 4), :])
        for c in range(1, NCH):
            for kk in range(4):
                nc.sync.dma_start(out=xt[c][:, bass.ts(kk, 4), :],
                                  in_=d_xt[:, bass.ts(kk, 4), bass.ts(c, CH)])

        # --- HAM warmup: ~100 tiny matmuls on a zeroed tile during the DMA
        # preamble dead time keep the PE busy >3.4us so the clock gate opens
        # to 8/8 (2.4 GHz) before the first real matmul issues.
        warm_src = work.tile([P, 64], BF16, tag="warm", bufs=1, name="warm_src")
        nc.vector.memset(warm_src[:], 0.0)
        warm_ps = psum.tile([P, CH], F32, tag="po", bufs=2, name="warm_ps")
        for _ in range(100):
            nc.tensor.matmul(warm_ps[0:64, 0:64], lhsT=warm_src[:], rhs=warm_src[:],
                             start=True, stop=True, skip_group_check=True)

        # --- emit helpers -------------------------------------------------
        def rope(src_ps, dst, c):
            # ACT copies the PSUM to bf16 SBUF (frees the bank fast), then the
            # rotate/scale runs all-bf16 on DVE at the 2x 16-bit rate
            cs = bass.ts(c, CH)
            raw = work.tile([P, CH], BF16, tag="raw", name="raw")
            nc.scalar.activation(raw[:], src_ps[:], mybir.ActivationFunctionType.Copy)
            t1 = work.tile([P, CH], BF16, tag="t1", name="t1")
            t2 = work.tile([P, CH], BF16, tag="t2", name="t2")
            nc.vector.tensor_mul(out=t1[:], in0=raw[:], in1=cos_sb[:, cs])
            for blk in range(4):
                lo = blk * 32
                swap_lo = (blk ^ 1) * 32
                # in0 stays in PSUM: cross-partition-offset operands are only
                # legal when one input is in PSUM (walrus SB-SB check)
                nc.vector.tensor_mul(out=t2[lo:lo + 32, :],
                                     in0=src_ps[swap_lo:swap_lo + 32, :],
                                     in1=sin_sb[lo:lo + 32, cs])
            nc.vector.tensor_add(out=dst[:, cs], in0=t1[:], in1=t2[:])

        pq_t, pk_t = {}, {}

        def qk_mms(c, k0, k1):
            if c not in pq_t:
                pq_t[c] = psum.tile([P, CH], F32, tag="pq", bufs=1, name=f"pq{c}")
                pk_t[c] = psum.tile([P, CH], F32, tag="pk", bufs=1, name=f"pk{c}")
            pq, pk = pq_t[c], pk_t[c]
            for k in range(k0, k1):
                nc.tensor.matmul(pq[:], lhsT=wq_sb[:, k, :], rhs=xt[c][:, k, :],
                                 start=(k == 0), stop=(k == NKT - 1), skip_group_check=True)
                nc.tensor.matmul(pk[:], lhsT=wk_sb[:, k, :], rhs=xt[c][:, k, :],
                                 start=(k == 0), stop=(k == NKT - 1), skip_group_check=True)
            if k1 == NKT:
                rope(pq, qt, c)
                rope(pk, kt, c)

        def v_fin(pv, t):
            nc.vector.tensor_copy(out=v_sb[t][:, 0:256], in_=pv[:, 0:256])
            nc.vector.tensor_copy(out=v_sb[t][:, VOFF:VOFF + 256], in_=pv[:, 256:512])
            nc.vector.memset(v_sb[t][:, 256:257], 1.0)
            nc.vector.memset(v_sb[t][:, VOFF + 256:VOFF + 257], 1.0)

        def v_single(t):
            c, tt = t // 4, t % 4
            pv = psum.tile([P, DV], F32, tag="pv", bufs=2, name=f"pv{t}")
            for k in range(NKT):
                nc.tensor.matmul(pv[:], lhsT=xt[c][:, k, bass.ts(tt, P)],
                                 rhs=wv_sb[:, k, :],
                                 start=(k == 0), stop=(k == NKT - 1), skip_group_check=True)
            v_fin(pv, t)

        def v_pair(c, tt0):
            v_single(4 * c + tt0)
            v_single(4 * c + tt0 + 1)

        e_tiles = {}

        def score_tile(c, t):
            cs0 = c * CH
            m = t - 4 * c
            off = max(m, 0) * P
            w = CH - off
            # both heads in one 2-bank PSUM tile: the pair of K=64 matmuls
            # lands adjacent (disjoint row halves -> concurrent in the array)
            # and a single paired exp halves the ACT per-op overhead
            ps_s = psum.tile([P, 2, CH], F32, tag="ps", bufs=1, name="ps_s")
            for h in range(2):
                nc.tensor.matmul(
                    ps_s[:, h, 0:w],
                    lhsT=kt[h * HD:(h + 1) * HD, bass.ts(t, P)],
                    rhs=qt[h * HD:(h + 1) * HD, bass.ds(cs0 + off, w)],
                    start=True, stop=True, skip_group_check=True)
            e = epool.tile([P, 2, CH], BF16, tag="e", bufs=24, name=f"e{c}_{t}")
            nc.scalar.activation(e[:, :, bass.ds(off, w)], ps_s[:, :, 0:w],
                                 mybir.ActivationFunctionType.Exp)
            if m >= 0:
                for h in range(2):
                    nc.vector.tensor_mul(out=e[:, h, bass.ts(m, P)],
                                         in0=e[:, h, bass.ts(m, P)], in1=tri_sb[:])
            e_tiles[(c, t)] = e

        def av_m(c, m):
            q_idx = 4 * c + m
            stage = outp.tile([P, 2, 257], BF16, tag="stage", name="stage")
            for h in range(2):
                po = psum.tile([P, CH], F32, tag="po", bufs=2, name="po")
                for t in range(q_idx + 1):
                    nc.tensor.matmul(
                        po[:, 0:257],
                        lhsT=e_tiles[(c, t)][:, h, bass.ts(m, P)],
                        rhs=v_sb[t][:, h * VOFF:h * VOFF + 257],
                        start=(t == 0), stop=(t == q_idx), skip_group_check=True)
                # stage raw numerator+denominator through SBUF; the softmax
                # division runs on host. ACT is free in the early chunks,
                # DVE in the late ones (ACT is then busy with exps).
                if c < 2:
                    nc.scalar.activation(stage[:, h, :], po[:, 0:257],
                                         mybir.ActivationFunctionType.Copy)
                else:
                    nc.vector.tensor_copy(out=stage[:, h, :], in_=po[:, 0:257])
                if q_idx == NST - 1:
                    # last q-tile: ship each half as soon as it is staged
                    nc.sync.dma_start(out=d_out[bass.ts(q_idx, P), h, :],
                                      in_=stage[:, h, :])
            if q_idx != NST - 1:
                nc.sync.dma_start(out=d_out[bass.ts(q_idx, P), :, :], in_=stage[:])

        # --- main emission sequence --------------------------------------
        qk_mms(0, 0, NKT)
        v_pair(0, 0)
        v_pair(0, 2)
        score_tile(0, 0)
        score_tile(0, 1)
        qk_mms(1, 0, 8)
        score_tile(0, 2)
        score_tile(0, 3)
        qk_mms(1, 8, NKT)
        for mm_ in range(4):
            av_m(0, mm_)
        v_pair(1, 0)
        v_pair(1, 2)
        qk_mms(2, 0, NKT)
        # scores c1 interleaved with av c1 (ACT-paced region)
        for t in range(5):
            score_tile(1, t)
        qk_mms(3, 0, 8)
        score_tile(1, 5)
        score_tile(1, 6)
        score_tile(1, 7)
        qk_mms(3, 8, NKT)
        av_m(1, 0)
        av_m(1, 1)
        av_m(1, 2)
        av_m(1, 3)
        # scores c2 interleaved with V chunk 2
        score_tile(2, 0)
        score_tile(2, 1)
        score_tile(2, 2)
        v_pair(2, 0)
        score_tile(2, 3)
        score_tile(2, 4)
        score_tile(2, 5)
        v_pair(2, 2)
        for t in range(6, 12):
            score_tile(2, t)
        # scores c3 (t0-11) interleaved with av c2
        score_tile(3, 0)
        score_tile(3, 1)
        score_tile(3, 2)
        av_m(2, 0)
        score_tile(3, 3)
        score_tile(3, 4)
        score_tile(3, 5)
        av_m(2, 1)
        score_tile(3, 6)
        score_tile(3, 7)
        score_tile(3, 8)
        av_m(2, 2)
        score_tile(3, 9)
        score_tile(3, 10)
        score_tile(3, 11)
        av_m(2, 3)
        # tail: V chunk 3 tiles interleaved with diag scores + av c3
        v_single(12)
        score_tile(3, 12)
        av_m(3, 0)
        v_single(13)
        score_tile(3, 13)
        av_m(3, 1)
        v_single(14)
        score_tile(3, 14)
        av_m(3, 2)
        v_single(15)
        score_tile(3, 15)
        av_m(3, 3)

    _legalize_waits(nc)
    _dedup_ldweights(nc)
    return nc


def _host_prep(hidden_states, position_ids, Wq, Wk, Wv):
    """Build the 8 per-core input maps."""
    hidden_states = np.asarray(hidden_states, dtype=np.float32)
    position_ids = np.asarray(position_ids)
    Wq = np.asarray(Wq, dtype=np.float32)
    Wk = np.asarray(Wk, dtype=np.float32)
    Wv = np.asarray(Wv, dtype=np.float32)

    scale = 1.0 / np.sqrt(HD)
    tri = np.triu(np.ones((P, P), dtype=np.float32)).astype(ml_dtypes.bfloat16)
    inv_freq = (1.0 / (THETA ** (np.arange(0, HD, 2, dtype=np.float32) / HD))).astype(np.float32)

    def _pkt(a):  # [HID, N] -> [P, NKT, N] (partition-major k-tiles)
        return np.ascontiguousarray(
            a.reshape(NKT, P, a.shape[1]).transpose(1, 0, 2))

    in_maps = []
    for core in range(8):
        b, p = core // 4, core % 4
        xt = _pkt(np.ascontiguousarray(hidden_states[b].T)).astype(ml_dtypes.bfloat16)
        wq = _pkt(Wq[:, p * P:(p + 1) * P] * scale).astype(ml_dtypes.bfloat16)
        wk = _pkt(Wk[:, p * P:(p + 1) * P]).astype(ml_dtypes.bfloat16)
        cols = []
        for h in (2 * p, 2 * p + 1):
            for r in range(G):
                j = r * HKV + h
                cols.append(Wv[:, j * HD:(j + 1) * HD])
        wv = _pkt(np.concatenate(cols, axis=1)).astype(ml_dtypes.bfloat16)

        pos = position_ids[b].astype(np.float32)
        freqs = pos[:, None] * inv_freq[None, :]          # [S, 32]
        cos32 = np.cos(freqs).T.astype(np.float32)        # [32, S]
        sin32 = np.sin(freqs).T.astype(np.float32)
        cost = np.ascontiguousarray(np.tile(cos32, (4, 1))).astype(ml_dtypes.bfloat16)
        sint = np.ascontiguousarray(
            np.tile(np.concatenate([-sin32, sin32], axis=0), (2, 1))).astype(ml_dtypes.bfloat16)

        in_maps.append({
            "xt": xt, "wq": wq, "wk": wk, "wv": wv,
            "cost": cost, "sint": sint, "tri": tri,
        })
    return in_maps


def kernel(hidden_states, position_ids, Wq, Wk, Wv):
    global LAST_RESULTS
    trace = bool(os.environ.get("CHEEMS_TRACE"))
    if trace:
        _install_ntff_hook()
    if "nc" not in _CACHE:
        _CACHE["nc"] = _build()
    nc = _CACHE["nc"]
    in_maps = _host_prep(hidden_states, position_ids, Wq, Wk, Wv)
    res = run_bass_kernel_spmd(nc, in_maps, core_ids=list(range(8)), trace=trace)
    LAST_RESULTS = res

    out = np.empty((B, S, HID), dtype=np.float32)
    for core in range(8):
        b, p = core // 4, core % 4
        raw = res.results[core]["out"].astype(np.float32)        # [S, 2, 257]
        core_out = raw[:, :, 0:256] / raw[:, :, 256:257]         # softmax denom
        for hl, h in enumerate((2 * p, 2 * p + 1)):
            for r in range(G):
                j = r * HKV + h
                out[b, :, j * HD:(j + 1) * HD] = core_out[:, hl, r * HD:(r + 1) * HD]
    return out.reshape(B, S, HID)


# revision 19
# speedup vs baseline: 1.0323x; 1.0006x over previous
"""Trainium2 Bass kernel for CheemsNonWoAttn (GQA attention block, no Wo).

Sharding: 8 cores = batch(2) x kv-head-pair(4). Each core handles one batch
element and 2 of the 8 kv heads (GQA: Q and K are repeated identically across
the 4 groups, so only 8 unique softmax matrices exist; V uses all 32 heads).

Per-core device program — 4-chunk pipeline over the sequence (chunks of 512):
  for each chunk c: Qt/Kt = Wq^T X_c^T, Wk^T X_c^T (PSUM), RoPE on DVE,
  V_c = X_c Wv, scores^T[k,q] = Kt^T Qt per head (row-half packed),
  E = exp(scores) on ACT, causal diag masked on DVE,
  out = E^T V with a ones-column producing the softmax denominator,
  denominator division staged through SBUF (ACT copy, DVE recip+scale).
Emission interleaves score tiles with projection matmuls so the PE stream
never stalls >~2us (keeps the HAM clock gate at 8/8 = 2.4 GHz).
"""
import os
import sys
import types
from contextlib import ExitStack

for _p in ("/opt/trn_rl_repo", "/root/.axon_site/_ro/trn_rl_repo"):
    if os.path.isdir(_p) and _p not in sys.path:
        sys.path.append(_p)

import numpy as np
import ml_dtypes

import concourse.bass as bass
import concourse.tile as tile
from concourse import mybir
from concourse.bass_utils import run_bass_kernel_spmd
from concourse.vector_clock import ScopedClock

# ---------------------------------------------------------------------------
# Patch 1: walrus rejects Drain instructions with >1 sync wait (CTRL ops have
# a single wait slot). Split the TileContext exit drain's waits across extra
# SP nops, one wait each.
def _patched_drain_and_barrier(self, tick_clock, wait_clock):
    nc = self.nc
    drain_bi = nc.sync.drain()
    wait_clock.add_sem_waits(drain_bi.ins, ScopedClock({None: tick_clock.global_clock}))
    inst = drain_bi.ins
    si = inst.sync_info
    if si is not None and si.on_wait is not None and len(si.on_wait) > 1:
        waits = list(si.on_wait)
        inst.sync_info = mybir.SyncInfo(
            on_wait=waits[:1],
            on_update=list(si.on_update) if si.on_update else [],
        )
        for w in waits[1:]:
            nbi = nc.sync.nop()
            nbi.ins.sync_info = mybir.SyncInfo(on_wait=[w], on_update=[])
    nc.all_engine_barrier()
    assert self.sems is not None
    popped = nc._tile_sem_poison_stack.pop()
    assert popped is self._sem_poison
    nc.clear_and_free_semaphores(list(self.sems.allocated().values()))
    nc.all_engine_barrier()


tile.TileContext._drain_and_barrier = _patched_drain_and_barrier


def _legalize_waits(nc):
    """This walrus build accepts at most one sync-wait per instruction.
    Split any instruction carrying N>1 waits into N-1 preceding same-engine
    nops (engines are in-order, so semantics are preserved)."""
    uid = 0
    for f in nc.m.functions:
        for blk in f.blocks:
            insts = list(blk.instructions)
            out, changed = [], False
            for inst in insts:
                si = getattr(inst, "sync_info", None)
                if si is not None and si.on_wait is not None and len(si.on_wait) > 1:
                    waits = list(si.on_wait)
                    for w in waits[:-1]:
                        uid += 1
                        out.append(mybir.InstNoOp(
                            name=f"{inst.name}_lw{uid}",
                            engine=inst.engine,
                            sync_info=mybir.SyncInfo(on_wait=[w], on_update=[]),
                            bass_nofuse=True,
                        ))
                    inst.sync_info = mybir.SyncInfo(
                        on_wait=waits[-1:],
                        on_update=list(si.on_update) if si.on_update else [],
                    )
                    changed = True
                out.append(inst)
            if changed:
                blk.instructions = out


def _dedup_ldweights(nc):
    """Clear the self-load on matmuls whose stationary operand AP is identical
    to the immediately preceding matmul in the final PE stream (walrus runs
    with ldw-opt disabled, so it reloads weights for every matmul otherwise)."""
    if os.environ.get("CHEEMS_NO_LDW_DEDUP"):
        return
    for f in nc.m.functions:
        for blk in f.blocks:
            prev_key = None
            for inst in blk.instructions:
                if not isinstance(inst, mybir.InstMatmult):
                    if isinstance(inst, mybir.InstLdweights):
                        prev_key = None
                    continue
                key = (repr(inst.ins[1]), inst.perf_mode, inst.is_transpose,
                       repr(inst.tile_position))
                if prev_key is not None and key == prev_key:
                    inst.ldweights = False
                prev_key = key


# Patch 2 (optional, for tracing): recreate the antenv.axon_hooks shim so
# run_bass_kernel_spmd(trace=True) can capture NTFF profiles under axon.
def _install_ntff_hook():
    try:
        if "antenv.axon_hooks" in sys.modules:
            return
        import antenv
        from trn_agent_boot.trn_boot import _ntff_profile_via_ctypes

        hook = _ntff_profile_via_ctypes("/opt/axon/libaxon_pjrt.so")
        mod = types.ModuleType("antenv.axon_hooks")
        mod._hook = hook
        mod.get_axon_ntff_profile_hook = lambda: mod._hook

        def _set(h):
            mod._hook = h

        mod.set_axon_ntff_profile_hook = _set
        sys.modules["antenv.axon_hooks"] = mod
        antenv.axon_hooks = mod
    except Exception:
        pass


# ---------------------------------------------------------------------------
B, S, HID = 2, 2048, 2048
NH, G = 32, 4
HD = 64          # head dim
HKV = 8          # kv heads
THETA = 10000.0
P = 128          # partitions
NKT = HID // P   # 16 k-tiles over the contraction dim
NST = S // P     # 16 s-tiles
NCH = 4          # s-chunks of 512
CH = 512
DV = 512         # v columns per core (8 heads x 64)
VROW = 528       # v tile row: [256 v | 1 one | 7 pad] x 2 heads
VOFF = 264

F32 = mybir.dt.float32
BF16 = mybir.dt.bfloat16

_CACHE = {}
LAST_RESULTS = None


def _build():
    nc = bass.Bass("TRN2")
    d_xt = nc.declare_dram_parameter("xt", [P, NKT, S], BF16, isOutput=False)
    d_wq = nc.declare_dram_parameter("wq", [P, NKT, P], BF16, isOutput=False)
    d_wk = nc.declare_dram_parameter("wk", [P, NKT, P], BF16, isOutput=False)
    d_wv = nc.declare_dram_parameter("wv", [P, NKT, DV], BF16, isOutput=False)
    d_cos = nc.declare_dram_parameter("cost", [P, S], BF16, isOutput=False)
    d_sin = nc.declare_dram_parameter("sint", [P, S], BF16, isOutput=False)
    d_tri = nc.declare_dram_parameter("tri", [P, P], BF16, isOutput=False)
    d_out = nc.declare_dram_parameter("out", [S, 2, 257], BF16, isOutput=True)

    with tile.TileContext(nc) as tc, ExitStack() as ctx:
        pers = ctx.enter_context(tc.tile_pool(name="pers", bufs=1))
        xpool = ctx.enter_context(tc.tile_pool(name="xpool", bufs=3))
        epool = ctx.enter_context(tc.tile_pool(name="epool", bufs=48))
        work = ctx.enter_context(tc.tile_pool(name="work", bufs=2))
        outp = ctx.enter_context(tc.tile_pool(name="outp", bufs=3))
        psum = ctx.enter_context(tc.tile_pool(name="psum", bufs=8, space="PSUM"))

        # --- persistent tiles
        wq_sb = pers.tile([P, NKT, P], BF16, tag="wq")
        wk_sb = pers.tile([P, NKT, P], BF16, tag="wk")
        wv_sb = pers.tile([P, NKT, DV], BF16, tag="wv")
        cos_sb = pers.tile([P, S], BF16, tag="cos")
        sin_sb = pers.tile([P, S], BF16, tag="sin")
        tri_sb = pers.tile([P, P], BF16, tag="tri")
        qt = pers.tile([P, S], BF16, tag="qt")
        kt = pers.tile([P, S], BF16, tag="kt")
        v_sb = [pers.tile([P, VROW], BF16, tag=f"v{t}", name=f"v{t}") for t in range(NST)]

        xt = [xpool.tile([P, NKT, CH], BF16, tag="xt", name=f"xt{c}") for c in range(NCH)]

        # --- DMA emission order (= data arrival order; see module docstring)
        # first pieces are tiny so the first matmuls start ~2us earlier,
        # interleaved so the QK k-loop is never starved of weights or xt
        nc.sync.dma_start(out=wq_sb[:, 0:2, :], in_=d_wq[:, 0:2, :])
        nc.sync.dma_start(out=xt[0][:, 0:2, :], in_=d_xt[:, 0:2, bass.ts(0, CH)])
        nc.sync.dma_start(out=wk_sb[:, 0:2, :], in_=d_wk[:, 0:2, :])
        nc.sync.dma_start(out=xt[0][:, 2:4, :], in_=d_xt[:, 2:4, bass.ts(0, CH)])
        nc.sync.dma_start(out=wq_sb[:, 2:8, :], in_=d_wq[:, 2:8, :])
        nc.sync.dma_start(out=wk_sb[:, 2:8, :], in_=d_wk[:, 2:8, :])
        nc.sync.dma_start(out=xt[0][:, 4:8, :], in_=d_xt[:, 4:8, bass.ts(0, CH)])
        nc.sync.dma_start(out=wq_sb[:, 8:16, :], in_=d_wq[:, 8:16, :])
        nc.sync.dma_start(out=wk_sb[:, 8:16, :], in_=d_wk[:, 8:16, :])
        nc.sync.dma_start(out=xt[0][:, 8:12, :], in_=d_xt[:, 8:12, bass.ts(0, CH)])
        nc.sync.dma_start(out=xt[0][:, 12:16, :], in_=d_xt[:, 12:16, bass.ts(0, CH)])
        nc.sync.dma_start(out=cos_sb[:], in_=d_cos[:])
        nc.sync.dma_start(out=sin_sb[:], in_=d_sin[:])
        nc.sync.dma_start(out=tri_sb[:], in_=d_tri[:])
        for kk in range(4):
            nc.sync.dma_start(out=wv_sb[:, bass.ts(kk, 4), :],
                              in_=d_wv[:, bass.ts(kk, 4), :])
        for c in range(1, NCH):
            for kk in range(4):
                nc.sync.dma_start(out=xt[c][:, bass.ts(kk, 4), :],
                                  in_=d_xt[:, bass.ts(kk, 4), bass.ts(c, CH)])

        # --- HAM warmup: ~100 tiny matmuls on a zeroed tile during the DMA
        # preamble dead time keep the PE busy >3.4us so the clock gate opens
        # to 8/8 (2.4 GHz) before the first real matmul issues.
        warm_src = work.tile([P, 64], BF16, tag="warm", bufs=1, name="warm_src")
        nc.vector.memset(warm_src[:], 0.0)
        warm_ps = psum.tile([P, CH], F32, tag="po", bufs=2, name="warm_ps")
        for _ in range(100):
            nc.tensor.matmul(warm_ps[0:64, 0:64], lhsT=warm_src[:], rhs=warm_src[:],
                             start=True, stop=True, skip_group_check=True)

        # --- emit helpers -------------------------------------------------
        def rope(src_ps, dst, c):
            # ACT copies the PSUM to bf16 SBUF (frees the bank fast), then the
            # rotate/scale runs all-bf16 on DVE at the 2x 16-bit rate
            cs = bass.ts(c, CH)
            raw = work.tile([P, CH], BF16, tag="raw", name="raw")
            nc.scalar.activation(raw[:], src_ps[:], mybir.ActivationFunctionType.Copy)
            t1 = work.tile([P, CH], BF16, tag="t1", name="t1")
            t2 = work.tile([P, CH], BF16, tag="t2", name="t2")
            nc.vector.tensor_mul(out=t1[:], in0=raw[:], in1=cos_sb[:, cs])
            for blk in range(4):
                lo = blk * 32
                swap_lo = (blk ^ 1) * 32
                # in0 stays in PSUM: cross-partition-offset operands are only
                # legal when one input is in PSUM (walrus SB-SB check)
                nc.vector.tensor_mul(out=t2[lo:lo + 32, :],
                                     in0=src_ps[swap_lo:swap_lo + 32, :],
                                     in1=sin_sb[lo:lo + 32, cs])
            nc.vector.tensor_add(out=dst[:, cs], in0=t1[:], in1=t2[:])

        pq_t, pk_t = {}, {}

        def qk_mms(c, k0, k1):
            if c not in pq_t:
                pq_t[c] = psum.tile([P, CH], F32, tag="pq", bufs=1, name=f"pq{c}")
                pk_t[c] = psum.tile([P, CH], F32, tag="pk", bufs=1, name=f"pk{c}")
            pq, pk = pq_t[c], pk_t[c]
            for k in range(k0, k1):
                nc.tensor.matmul(pq[:], lhsT=wq_sb[:, k, :], rhs=xt[c][:, k, :],
                                 start=(k == 0), stop=(k == NKT - 1), skip_group_check=True)
                nc.tensor.matmul(pk[:], lhsT=wk_sb[:, k, :], rhs=xt[c][:, k, :],
                                 start=(k == 0), stop=(k == NKT - 1), skip_group_check=True)
            if k1 == NKT:
                rope(pq, qt, c)
                rope(pk, kt, c)

        def v_fin(pv, t):
            nc.vector.tensor_copy(out=v_sb[t][:, 0:256], in_=pv[:, 0:256])
            nc.vector.tensor_copy(out=v_sb[t][:, VOFF:VOFF + 256], in_=pv[:, 256:512])
            nc.vector.memset(v_sb[t][:, 256:257], 1.0)
            nc.vector.memset(v_sb[t][:, VOFF + 256:VOFF + 257], 1.0)

        def v_single(t):
            c, tt = t // 4, t % 4
            pv = psum.tile([P, DV], F32, tag="pv", bufs=2, name=f"pv{t}")
            for k in range(NKT):
                nc.tensor.matmul(pv[:], lhsT=xt[c][:, k, bass.ts(tt, P)],
                                 rhs=wv_sb[:, k, :],
                                 start=(k == 0), stop=(k == NKT - 1), skip_group_check=True)
            v_fin(pv, t)

        def v_pair(c, tt0):
            v_single(4 * c + tt0)
            v_single(4 * c + tt0 + 1)

        e_tiles = {}

        def score_tile(c, t):
            cs0 = c * CH
            m = t - 4 * c
            off = max(m, 0) * P
            w = CH - off
            # both heads in one 2-bank PSUM tile: the pair of K=64 matmuls
            # lands adjacent (disjoint row halves -> concurrent in the array)
            # and a single paired exp halves the ACT per-op overhead
            ps_s = psum.tile([P, 2, CH], F32, tag="ps", bufs=1, name="ps_s")
            for h in range(2):
                nc.tensor.matmul(
                    ps_s[:, h, 0:w],
                    lhsT=kt[h * HD:(h + 1) * HD, bass.ts(t, P)],
                    rhs=qt[h * HD:(h + 1) * HD, bass.ds(cs0 + off, w)],
                    start=True, stop=True, skip_group_check=True)
            e = epool.tile([P, 2, CH], BF16, tag="e", bufs=24, name=f"e{c}_{t}")
            nc.scalar.activation(e[:, :, bass.ds(off, w)], ps_s[:, :, 0:w],
                                 mybir.ActivationFunctionType.Exp)
            if m >= 0:
                for h in range(2):
                    nc.vector.tensor_mul(out=e[:, h, bass.ts(m, P)],
                                         in0=e[:, h, bass.ts(m, P)], in1=tri_sb[:])
            e_tiles[(c, t)] = e

        def av_m(c, m):
            q_idx = 4 * c + m
            stage = outp.tile([P, 2, 257], BF16, tag="stage", name="stage")
            for h in range(2):
                po = psum.tile([P, CH], F32, tag="po", bufs=2, name="po")
                for t in range(q_idx + 1):
                    nc.tensor.matmul(
                        po[:, 0:257],
                        lhsT=e_tiles[(c, t)][:, h, bass.ts(m, P)],
                        rhs=v_sb[t][:, h * VOFF:h * VOFF + 257],
                        start=(t == 0), stop=(t == q_idx), skip_group_check=True)
                # stage raw numerator+denominator through SBUF; the softmax
                # division runs on host. ACT is free in the early chunks,
                # DVE in the late ones (ACT is then busy with exps).
                if c < 2:
                    nc.scalar.activation(stage[:, h, :], po[:, 0:257],
                                         mybir.ActivationFunctionType.Copy)
                else:
                    nc.vector.tensor_copy(out=stage[:, h, :], in_=po[:, 0:257])
                if q_idx == NST - 1:
                    # last q-tile: ship each half as soon as it is staged
                    nc.sync.dma_start(out=d_out[bass.ts(q_idx, P), h, :],
                                      in_=stage[:, h, :])
            if q_idx != NST - 1:
                nc.sync.dma_start(out=d_out[bass.ts(q_idx, P), :, :], in_=stage[:])

        # --- main emission sequence --------------------------------------
        qk_mms(0, 0, NKT)
        v_pair(0, 0)
        v_pair(0, 2)
        score_tile(0, 0)
        score_tile(0, 1)
        qk_mms(1, 0, 8)
        score_tile(0, 2)
        score_tile(0, 3)
        qk_mms(1, 8, NKT)
        for mm_ in range(4):
            av_m(0, mm_)
        v_pair(1, 0)
        v_pair(1, 2)
        qk_mms(2, 0, NKT)
        # scores c1 interleaved with av c1 (ACT-paced region)
        for t in range(5):
            score_tile(1, t)
        qk_mms(3, 0, 8)
        score_tile(1, 5)
        score_tile(1, 6)
        score_tile(1, 7)
        qk_mms(3, 8, NKT)
        av_m(1, 0)
        av_m(1, 1)
        av_m(1, 2)
        av_m(1, 3)
        # scores c2 interleaved with V chunk 2
        score_tile(2, 0)
        score_tile(2, 1)
        score_tile(2, 2)
        v_pair(2, 0)
        score_tile(2, 3)
        score_tile(2, 4)
        score_tile(2, 5)
        v_pair(2, 2)
        for t in range(6, 12):
            score_tile(2, t)
        # scores c3 (t0-11) interleaved with av c2
        score_tile(3, 0)
        score_tile(3, 1)
        score_tile(3, 2)
        av_m(2, 0)
        score_tile(3, 3)
        score_tile(3, 4)
        score_tile(3, 5)
        av_m(2, 1)
        score_tile(3, 6)
        score_tile(3, 7)
        score_tile(3, 8)
        av_m(2, 2)
        score_tile(3, 9)
        score_tile(3, 10)
        score_tile(3, 11)
        av_m(2, 3)
        # tail: V chunk 3 tiles interleaved with diag scores + av c3
        v_single(12)
        score_tile(3, 12)
        av_m(3, 0)
        v_single(13)
        score_tile(3, 13)
        av_m(3, 1)
        v_single(14)
        score_tile(3, 14)
        av_m(3, 2)
        v_single(15)
        score_tile(3, 15)
        av_m(3, 3)

    _legalize_waits(nc)
    _dedup_ldweights(nc)
    return nc


def _host_prep(hidden_states, position_ids, Wq, Wk, Wv):
    """Build the 8 per-core input maps."""
    hidden_states = np.asarray(hidden_states, dtype=np.float32)
    position_ids = np.asarray(position_ids)
    Wq = np.asarray(Wq, dtype=np.float32)
    Wk = np.asarray(Wk, dtype=np.float32)
    Wv = np.asarray(Wv, dtype=np.float32)

    scale = 1.0 / np.sqrt(HD)
    tri = np.triu(np.ones((P, P), dtype=np.float32)).astype(ml_dtypes.bfloat16)
    inv_freq = (1.0 / (THETA ** (np.arange(0, HD, 2, dtype=np.float32) / HD))).astype(np.float32)

    def _pkt(a):  # [HID, N] -> [P, NKT, N] (partition-major k-tiles)
        return np.ascontiguousarray(
            a.reshape(NKT, P, a.shape[1]).transpose(1, 0, 2))

    in_maps = []
    for core in range(8):
        b, p = core // 4, core % 4
        xt = _pkt(np.ascontiguousarray(hidden_states[b].T)).astype(ml_dtypes.bfloat16)
        wq = _pkt(Wq[:, p * P:(p + 1) * P] * scale).astype(ml_dtypes.bfloat16)
        wk = _pkt(Wk[:, p * P:(p + 1) * P]).astype(ml_dtypes.bfloat16)
        cols = []
        for h in (2 * p, 2 * p + 1):
            for r in range(G):
                j = r * HKV + h
                cols.append(Wv[:, j * HD:(j + 1) * HD])
        wv = _pkt(np.concatenate(cols, axis=1)).astype(ml_dtypes.bfloat16)

        pos = position_ids[b].astype(np.float32)
        freqs = pos[:, None] * inv_freq[None, :]          # [S, 32]
        cos32 = np.cos(freqs).T.astype(np.float32)        # [32, S]
        sin32 = np.sin(freqs).T.astype(np.float32)
        cost = np.ascontiguousarray(np.tile(cos32, (4, 1))).astype(ml_dtypes.bfloat16)
        sint = np.ascontiguousarray(
            np.tile(np.concatenate([-sin32, sin32], axis=0), (2, 1))).astype(ml_dtypes.bfloat16)

        in_maps.append({
            "xt": xt, "wq": wq, "wk": wk, "wv": wv,
            "cost": cost, "sint": sint, "tri": tri,
        })
    return in_maps


def kernel(hidden_states, position_ids, Wq, Wk, Wv):
    global LAST_RESULTS
    trace = bool(os.environ.get("CHEEMS_TRACE"))
    if trace:
        _install_ntff_hook()
    if "nc" not in _CACHE:
        _CACHE["nc"] = _build()
    nc = _CACHE["nc"]
    in_maps = _host_prep(hidden_states, position_ids, Wq, Wk, Wv)
    res = run_bass_kernel_spmd(nc, in_maps, core_ids=list(range(8)), trace=trace)
    LAST_RESULTS = res

    out = np.empty((B, S, HID), dtype=np.float32)
    for core in range(8):
        b, p = core // 4, core % 4
        raw = res.results[core]["out"].astype(np.float32)        # [S, 2, 257]
        core_out = raw[:, :, 0:256] / raw[:, :, 256:257]         # softmax denom
        for hl, h in enumerate((2 * p, 2 * p + 1)):
            for r in range(G):
                j = r * HKV + h
                out[b, :, j * HD:(j + 1) * HD] = core_out[:, hl, r * HD:(r + 1) * HD]
    return out.reshape(B, S, HID)
